# revision 1
# baseline (speedup 1.0000x reference)
"""CutOut kernel for Trainium2 (Bass/Tile), data-parallel over 8 NeuronCores.

Problem: images [64, 512, 512, 3] f32; per-sample integer centers (cy, cx);
length 50. Output = images with the (clipped) 50x50 square at each sample's
center set to 0.0.

Strategy:
  - Shard batch 64 -> 8 samples per core (pure data parallel).
  - Host precomputes per-sample keep masks from the centers (tiny int math):
      rowkeep f32 [128, 32]  -- element [p, s*4+t] = keep flag of row t*128+p
      colkeep bf16 [8, 128, 1536] -- per-sample column keep, broadcast across
      the 128 partitions (DVE lanes can only read their own partition).
    Masks are *data*, so the compiled NEFF is value-independent.
  - Device, per (sample, row-tile of 128 rows):
      load img tile [128, 1536] f32;
      img = (colkeep max rowkeep_scalar) * img   (one fused DVE op);
      store tile.
    Mask values are exactly 0.0/1.0 => output is bit-exact.
"""

import numpy as np
import ml_dtypes

B, H, W, C = 64, 512, 512, 3
N_CORES = 8
BPC = B // N_CORES  # samples per core
WC = W * C  # 1536 floats per image row
NT = H // 128  # row-tiles per sample

_nc_cache = None


def _build_bass(repeat=1):
    from contextlib import ExitStack

    import concourse.bass as bass
    import concourse.mybir as mybir

    nc = bass.Bass("TRN2", target_bir_lowering=False, debug=False)
    img = nc.dram_tensor("img", [BPC, H, WC], mybir.dt.float32, kind="ExternalInput")
    ckb = nc.dram_tensor(
        "ckb", [128, BPC * WC], mybir.dt.bfloat16, kind="ExternalInput"
    )
    rkt = nc.dram_tensor(
        "rkt", [128, BPC * NT], mybir.dt.float32, kind="ExternalInput"
    )
    out = nc.dram_tensor("out", [BPC, H, WC], mybir.dt.float32, kind="ExternalOutput")

    img_ap = img.ap()
    out_ap = out.ap()

    NTILES = BPC * NT  # 32 tiles of [128, WC] per core
    NBUF = 8

    # Raw bass (no Tile scheduler): hand-built 3-stage pipeline with one
    # semaphore wait per instruction (the ISA limit for compute structs).
    #   SP  (nc.sync):   mask loads, image loads (ring-FIFO, slot-gated)
    #   DVE (nc.vector): 32 keep-mask builds, then 32 in-place multiplies
    #   ACT (nc.scalar): stores (HWDGE on the Activation ring)
    with ExitStack() as ctx:
        masksem = ctx.enter_context(nc.semaphore("masksem"))
        loadsem = ctx.enter_context(nc.semaphore("loadsem"))
        storesem = ctx.enter_context(nc.semaphore("storesem"))
        dvesem = ctx.enter_context(nc.semaphore("dvesem"))
        rk = ctx.enter_context(
            nc.sbuf_tensor("rk", [128, BPC * NT], mybir.dt.float32)
        )
        ckall = ctx.enter_context(
            nc.sbuf_tensor("ckall", [128, BPC * WC], mybir.dt.bfloat16)
        )
        keeps = [
            ctx.enter_context(
                nc.sbuf_tensor(f"keep{i}", [128, WC], mybir.dt.bfloat16)
            )
            for i in range(NTILES)
        ]
        bufs = [
            ctx.enter_context(
                nc.sbuf_tensor(f"buf{i}", [128, WC], mybir.dt.float32)
            )
            for i in range(NBUF)
        ]

        # --- SP: image loads (start immediately at t=0) ---
        for r in range(repeat):
            for k in range(NTILES):
                i = r * NTILES + k
                s, t = divmod(k, NT)
                b = i % NBUF
                ld = nc.sync.dma_start(
                    bufs[b][:, :], img_ap[s, t * 128 : (t + 1) * 128, :]
                )
                if i >= NBUF:
                    # slot reuse: wait until store of tile i-NBUF drained
                    ld.wait_op(storesem, 16 * (i - NBUF + 1), "sem-ge")
                ld.then_inc(loadsem, 16)
        # program completion gate: all stores landed
        nc.sync.wait_ge(storesem, 16 * NTILES * repeat)

        # --- ACT: mask loads (ring idle early), then stores ---
        nc.scalar.dma_start(rk[:, :], rkt.ap()).then_inc(masksem, 16)
        ckb_ap = ckb.ap()
        for s in range(BPC):
            nc.scalar.dma_start(
                ckall[:, s * WC : (s + 1) * WC],
                ckb_ap[:, s * WC : (s + 1) * WC],
            ).then_inc(masksem, 16)

        # --- DVE: keep-mask build interleaved with in-place multiplies ---
        for r in range(repeat):
            for k in range(NTILES):
                i = r * NTILES + k
                b = i % NBUF
                if r == 0:
                    s, t = divmod(k, NT)
                    ts = nc.vector.tensor_scalar_max(
                        keeps[k][:, :],
                        ckall[:, s * WC : (s + 1) * WC],
                        rk[:, k : k + 1],
                    )
                    if t == 0:
                        # rk + col-mask chunks 0..s landed
                        ts.wait_op(masksem, 16 * (s + 2), "sem-ge")
                tt = nc.vector.tensor_mul(
                    bufs[b][:, :], bufs[b][:, :], keeps[k][:, :]
                )
                tt.wait_op(loadsem, 16 * (i + 1), "sem-ge")
                tt.then_inc(dvesem, 1)

        # --- ACT: stores ---
        for r in range(repeat):
            for k in range(NTILES):
                i = r * NTILES + k
                s, t = divmod(k, NT)
                b = i % NBUF
                st = nc.scalar.dma_start(
                    out_ap[s, t * 128 : (t + 1) * 128, :], bufs[b][:, :]
                )
                st.wait_op(dvesem, i + 1, "sem-ge")
                st.then_inc(storesem, 16)
    return nc


def _get_nc():
    global _nc_cache
    if _nc_cache is None:
        _nc_cache = _build_bass()
    return _nc_cache


def _host_masks(center_y, center_x, length):
    """Per-sample row/col keep masks (1.0 = keep, 0.0 = cut), f32/bf16."""
    half = int(length) // 2
    cy = center_y.astype(np.int64)
    cx = center_x.astype(np.int64)
    rows = np.arange(H, dtype=np.int64)
    cols = np.arange(W, dtype=np.int64)
    row_in = (rows[None, :] >= cy[:, None] - half) & (rows[None, :] < cy[:, None] + half)
    col_in = (cols[None, :] >= cx[:, None] - half) & (cols[None, :] < cx[:, None] + half)
    rowkeep = (~row_in).astype(np.float32)  # [B, H]
    colkeep = (~col_in).astype(np.float32)  # [B, W]
    colkeep = np.repeat(colkeep, C, axis=1)  # [B, W*C]
    return rowkeep, colkeep


def kernel(images, center_y, center_x, length):
    from concourse.bass_utils import run_bass_kernel_spmd

    images = np.asarray(images)
    out_dtype = images.dtype
    rowkeep, colkeep = _host_masks(np.asarray(center_y), np.asarray(center_x), length)

    imgs = np.ascontiguousarray(images.reshape(B, H, WC), dtype=np.float32)
    colkeep_b = colkeep.astype(ml_dtypes.bfloat16)  # exact for 0.0 / 1.0

    in_maps = []
    for c in range(N_CORES):
        sl = slice(c * BPC, (c + 1) * BPC)
        # [128, BPC*NT]: element [p, s*4+t] = rowkeep[c*BPC+s, t*128+p]
        rk = rowkeep[sl].reshape(BPC, NT, 128).transpose(2, 0, 1).reshape(128, BPC * NT)
        # [128, BPC*WC]: every partition holds all samples' col masks
        ck = np.broadcast_to(colkeep_b[sl].reshape(1, BPC * WC), (128, BPC * WC))
        in_maps.append(
            {
                "img": imgs[sl],
                "ckb": np.ascontiguousarray(ck),
                "rkt": np.ascontiguousarray(rk),
            }
        )

    nc = _get_nc()
    res = run_bass_kernel_spmd(nc, in_maps, core_ids=list(range(N_CORES)))
    full = np.concatenate([r["out"].reshape(BPC, H, W, C) for r in res.results], axis=0)
    return full.astype(out_dtype, copy=False)

